# revision 27
# baseline (speedup 1.0000x reference)
"""Distributed Trainium2 kernel for a 2-layer GCN + ragged segment-mean pooling.

reference math:
    z1 = h @ W1 + b1;  h1 = tanh(segment_sum(z1[edge_src], edge_dst, N))
    z2 = h1 @ W2 + b2; h2 = tanh(segment_sum(z2[edge_src], edge_dst, N))
    out[s] = mean over nodes with seg_ids==s of h2  -> [B, MC, H]

Sharding: nodes split contiguously over 8 cores; weights replicated.
Per layer: local z matmul (bf16, PE) -> AllGather(z) -> per-core edge
gather (dma_gather, dst-sorted edges in 128-chunks, int16 idx against
4 quarter bases) -> one-hot matmul scatter-add into PSUM -> tanh.
Pooling: seg-sorted node gather + one-hot matmul into 800 slots,
ReduceScatter(add), scale by host-precomputed 1/count.
"""
import math
import sys
import types

import numpy as np

sys.path.insert(0, "/opt/trn_rl_repo")
if "/root/.axon_site" not in sys.path:
    sys.path.insert(0, "/root/.axon_site")

import concourse.bass as bass
import concourse.bacc as bacc
import concourse.tile as tile
from concourse.tile import add_dep_helper
import concourse.mybir as mybir
from concourse.bass_utils import run_bass_kernel_spmd

BF16 = mybir.dt.float16  # compute dtype (fp16: 10-bit mantissa, values O(1))
F32 = mybir.dt.float32
I16 = mybir.dt.int16
bf16_np = mybir.dt.np(BF16)

NC = 8          # cores
P = 128         # partitions / tile width
QB = 32768      # int16-index quarter size
ST = 3          # node tiles per supertile (2 supertiles in flight = 6 PSUM banks)
MAX_CALL_CH = 4  # chunks per dma_gather call (512 idxs)
N_SWQ = 4       # SWDGE queues to rotate over


def _install_ntff_hook():
    try:
        import antenv
        if getattr(antenv, "axon_hooks", None) is not None:
            return
        mod = types.ModuleType("antenv.axon_hooks")
        _hook = [None]
        mod.set_axon_ntff_profile_hook = lambda h: _hook.__setitem__(0, h)
        mod.get_axon_ntff_profile_hook = lambda: _hook[0]
        sys.modules["antenv.axon_hooks"] = mod
        antenv.axon_hooks = mod
        from trn_agent_boot.trn_boot import _ntff_profile_via_ctypes
        mod.set_axon_ntff_profile_hook(
            _ntff_profile_via_ctypes("/opt/axon/libaxon_pjrt.so")
        )
    except Exception:
        pass


def _axon_reset():
    try:
        import ctypes
        import time
        lib = ctypes.CDLL("/opt/axon/libaxon_pjrt.so")
        lib.axon_reset.restype = ctypes.c_int64
        lib.axon_reset()
        time.sleep(3)
    except Exception:
        pass


def _wrap_idx(stream_i16: np.ndarray) -> np.ndarray:
    """[TOT] int16 -> [128, TOT//16] wrapped in 16 partitions, replicated x8."""
    w = stream_i16.reshape(-1, 16).T  # [16, TOT/16]
    return np.tile(w, (8, 1)).astype(np.int16)


class _Sched:
    """Static (core-independent) schedule shared by the SPMD graph."""


def _placement(N, src, dst):
    """Assign nodes to (core, tile, slot) positions: balance in-degree per
    tile and out-degree mass per quarter group. Returns (pos[node], NT)."""
    E = len(src)
    npcn = N // NC
    base = max(math.ceil(npcn / P), int(round(E / (NC * 4 * 463.0))))
    NT = 4 * math.ceil(base / 4)
    T_ALL = NC * NT
    indeg = np.bincount(dst, minlength=N)
    order = np.argsort(-indeg, kind="stable")
    # snake round-robin over all tiles
    rounds = math.ceil(N / T_ALL)
    fwd = np.arange(T_ALL)
    tile_seq = np.concatenate(
        [fwd if r % 2 == 0 else fwd[::-1] for r in range(rounds)])[:N]
    gtile = np.empty(N, dtype=np.int64)   # global tile per node
    gtile[order] = tile_seq
    slot = np.empty(N, dtype=np.int64)
    srt = np.argsort(gtile, kind="stable")
    starts = np.searchsorted(gtile[srt], np.arange(T_ALL))
    slot[srt] = np.arange(N) - starts[gtile[srt]]
    assert slot.max() < P
    # quarter grouping per core: snake tiles by out-degree mass into 4 groups
    outdeg = np.bincount(src, minlength=N).astype(np.int64)
    tile_mass = np.bincount(gtile, weights=outdeg, minlength=T_ALL)
    new_tile = np.empty(T_ALL, dtype=np.int64)
    GQ = NT // 4
    for c in range(NC):
        tl = np.arange(c * NT, (c + 1) * NT)
        morder = np.argsort(-tile_mass[tl], kind="stable")
        grp_fill = np.zeros(4, dtype=np.int64)
        for i, ti in enumerate(morder):
            g = i % 8
            g = g if g < 4 else 7 - g
            new_tile[tl[ti]] = c * NT + g * GQ + grp_fill[g]
            grp_fill[g] += 1
    gtile2 = new_tile[gtile]
    pos = gtile2 * P + slot
    return pos, NT


def _preprocess(h, W1, b1, W2, b2, edge_src, edge_dst, seg_ids, n_slots):
    N, D = h.shape
    assert N % NC == 0
    src0 = edge_src.astype(np.int64)
    dst0 = edge_dst.astype(np.int64)
    seg0 = seg_ids.astype(np.int64)

    npos, NT = _placement(N, src0, dst0)
    NPC = NT * P              # position slots per core
    NQ = 4
    CPR = NPC // 4            # rows per rank per AG chunk
    NPCP = NPC
    SPC = n_slots // NC
    PST = math.ceil(n_slots / P)     # pool slot tiles

    src = npos[src0]
    dst = npos[dst0]

    # ---- edge schedule: runs keyed by (core, tile, quarter) ----
    core = dst // NPC
    t_loc = (dst - core * NPC) // P
    src_rank = src // NPC
    src_loc = src - src_rank * NPC
    q = src_loc // CPR
    key = (core * NT + t_loc) * NQ + q
    order = np.argsort(key, kind="stable")
    skey = key[order]
    ssrc = src[order]
    sdst = dst[order]
    counts = np.bincount(key, minlength=NC * NT * NQ).reshape(NC, NT, NQ)
    nch = (counts.max(axis=0) + P - 1) // P          # [NT, NQ] chunks
    empty = nch.sum(axis=1) == 0
    nch[empty, 0] = 1                                 # every tile gets >=1 chunk

    # stream order: wave emission — st[q0..q2], then (st-1)[q3] deferred
    NST = math.ceil(NT / ST)
    run_order = []
    for st in range(NST):
        for qq in range(min(NQ, 3)):
            run_order.append((st, qq))
        if st >= 1 and NQ == 4:
            run_order.append((st - 1, 3))
    if NQ == 4:
        run_order.append((NST - 1, 3))
    calls = []       # (q, chunk_off, n_chunks) per gather call
    chunk_tile = []  # owning node-tile per chunk, stream order
    chunk_seq = [[[] for _ in range(NQ)] for _ in range(NT)]  # (t,q) -> stream chunk ids
    run_call_end = {}  # (st,q) -> index past the run's last call
    off = 0
    for (st, qq) in run_order:
        tiles = range(st * ST, min((st + 1) * ST, NT))
        call_off = off
        tl = list(tiles)
        for p0 in range(0, len(tl), 2):
            pair = tl[p0:p0 + 2]
            mx = max((int(nch[t, qq]) for t in pair), default=0)
            for k in range(mx):
                for t in pair:
                    if k < nch[t, qq]:
                        chunk_seq[t][qq].append(off)
                        chunk_tile.append(t)
                        off += 1
        co = call_off
        while co < off:
            n = min(MAX_CALL_CH, off - co)
            calls.append((qq, co, n))
            co += n
        run_call_end[(st, qq)] = len(calls)
    TOTCH = off
    # AG2 chunk k triggers after the run that flushes its last z2 tile
    # (q3 of the covering supertile), +2 calls of slack
    ag2_after_call = {}
    if NQ == 4:
        for k in range(4):
            t_last = (min((k + 1) * CPR, NPC) - 1) // P
            st_k = min(t_last // ST, NST - 1)
            ci = min(run_call_end[(st_k, 3)] + 2, len(calls))
            ag2_after_call.setdefault(ci, []).append(k)
    else:
        ag2_after_call = {}
    # AG1 chunk k (k>=1) triggers just before its first consumer run
    ag1_after_call = {}
    if NQ == 4:
        ag1_after_call.setdefault(run_call_end[(0, 0)], []).append(1)
        ag1_after_call.setdefault(run_call_end[(0, 1)], []).append(2)
        ag1_after_call.setdefault(run_call_end[(1, 2)], []).append(3)
    TOT = TOTCH * P
    chunk_tile = np.asarray(chunk_tile)
    # start/stop chunk per tile
    first_chunk = np.full(NT, -1, dtype=np.int64)
    last_chunk = np.full(NT, -1, dtype=np.int64)
    for ci, t in enumerate(chunk_tile):
        if first_chunk[t] < 0:
            first_chunk[t] = ci
        last_chunk[t] = ci

    # ---- per-core padded streams ----
    # chunk-id table [NT, NQ, max_nch] -> stream chunk id
    max_nch = int(nch.max())
    chunk_id_tab = np.zeros((NT, NQ, max_nch), dtype=np.int64)
    for t in range(NT):
        for qq in range(NQ):
            for k, cid in enumerate(chunk_seq[t][qq]):
                chunk_id_tab[t, qq, k] = cid
    group_start = np.searchsorted(skey, np.arange(NC * NT * NQ), side="left")
    rank_in_run = np.arange(len(skey)) - group_start[skey]
    e_t = (sdst - (sdst // NPC) * NPC) // P
    e_rank = ssrc // NPC
    e_loc = ssrc - e_rank * NPC
    e_q = e_loc // CPR
    e_core = sdst // NPC
    pos = chunk_id_tab[e_t, e_q, rank_in_run // P] * P + rank_in_run % P

    idx_stream = np.zeros((NC, TOT), dtype=np.int16)
    dst_stream = np.full((NC, TOT), -1.0, dtype=np.float32)
    idx_stream[e_core, pos] = (e_rank * CPR + e_loc - e_q * CPR).astype(np.int16)
    dst_stream[e_core, pos] = (sdst - e_core * NPC - e_t * P).astype(np.float32)

    # ---- pooling schedule: runs keyed by (core, slot_tile) ----
    seg = seg0
    ncore = npos // NPC
    stile = seg // P
    pkey = ncore * PST + stile
    porder = np.argsort(pkey, kind="stable")
    pskey = pkey[porder]
    pseg = seg[porder]
    pnode_loc = (npos - ncore * NPC)[porder]
    pcounts = np.bincount(pkey, minlength=NC * PST).reshape(NC, PST)
    pnch = (pcounts.max(axis=0) + P - 1) // P
    pnch[pnch == 0] = 1
    prun_off = np.zeros(PST, dtype=np.int64)
    poff = 0
    pchunk_tile = []
    for s in range(PST):
        prun_off[s] = poff
        pchunk_tile.extend([s] * int(pnch[s]))
        poff += int(pnch[s])
    PTOTCH = poff
    PTOT = PTOTCH * P
    pchunk_tile = np.asarray(pchunk_tile)

    pgroup_start = np.searchsorted(pskey, np.arange(NC * PST), side="left")
    prank = np.arange(len(pskey)) - pgroup_start[pskey]
    p_core = pskey // PST
    p_s = pskey % PST
    ppos = prun_off[p_s] * P + prank

    pidx_stream = np.zeros((NC, PTOT), dtype=np.int16)
    pdst_stream = np.full((NC, PTOT), -1.0, dtype=np.float32)
    pidx_stream[p_core, ppos] = pnode_loc.astype(np.int16)
    pdst_stream[p_core, ppos] = (pseg - p_s * P).astype(np.float32)

    cnts = np.bincount(seg, minlength=n_slots).astype(np.float32)
    inv = 1.0 / np.maximum(cnts, 1.0)

    # ---- host-side tensors per core ----
    h_pos = np.zeros((NC * NPC, D), dtype=np.float32)
    h_pos[npos] = h
    hbf = h_pos.astype(bf16_np)
    hTc3 = np.ascontiguousarray(
        hbf.reshape(NC, NPC, D)[:, 3 * CPR:4 * CPR, :]
        .reshape(NC * CPR, D).T)
    iota = np.tile(np.arange(P, dtype=np.float32), (P, 1)).astype(bf16_np)
    ones = np.ones((1, P), dtype=np.float32).astype(bf16_np)
    ident = np.eye(P, dtype=np.float32).astype(bf16_np)
    in_maps = []
    for c in range(NC):
        in_maps.append({
            "hT0": np.ascontiguousarray(hbf[c * NPC:(c + 1) * NPC].T),
            "W1": W1.astype(bf16_np),
            "W2": W2.astype(bf16_np),
            "b1": b1.reshape(1, D).astype(bf16_np),
            "b2": b2.reshape(1, D).astype(bf16_np),
            "iota": iota,
            "ones": ones,
            "ident": ident,
            "hTc3": hTc3,
            "invc": inv[c * SPC:(c + 1) * SPC].reshape(SPC, 1),
            "idxw": _wrap_idx(idx_stream[c]),
            "dstw": np.ascontiguousarray(
                dst_stream[c].reshape(TOTCH, P).T.astype(bf16_np)),
            "pidxw": _wrap_idx(pidx_stream[c]),
            "pdstw": np.ascontiguousarray(
                pdst_stream[c].reshape(PTOTCH, P).T.astype(bf16_np)),
        })

    s = _Sched()
    s.N, s.D, s.NPC, s.NT, s.NQ, s.NPCP = N, D, NPC, NT, NQ, NPCP
    s.n_slots, s.SPC, s.PST = n_slots, SPC, PST
    s.NST = NST
    s.nch, s.calls, s.chunk_tile = nch, calls, chunk_tile
    s.ag2_after_call = ag2_after_call if NQ == 4 else {}
    s.ag1_after_call = ag1_after_call if NQ == 4 else {}
    s.first_chunk, s.last_chunk = first_chunk, last_chunk
    s.TOTCH, s.TOT = TOTCH, TOT
    s.pnch, s.pchunk_tile, s.PTOTCH, s.PTOT = pnch, pchunk_tile, PTOTCH, PTOT
    s.prun_off = prun_off
    s.max_call_ch = max(n for (_, _, n) in calls)
    s.CPR = CPR
    return s, in_maps


def _build(s):
    D = s.D
    nc = bacc.Bacc("TRN2", target_bir_lowering=False, debug=False,
                   num_swdge_queues=N_SWQ)

    hT0 = nc.declare_dram_parameter("hT0", [D, s.NPC], BF16, isOutput=False)
    W1 = nc.declare_dram_parameter("W1", [D, D], BF16, isOutput=False)
    W2 = nc.declare_dram_parameter("W2", [D, D], BF16, isOutput=False)
    b1 = nc.declare_dram_parameter("b1", [1, D], BF16, isOutput=False)
    b2 = nc.declare_dram_parameter("b2", [1, D], BF16, isOutput=False)
    iota = nc.declare_dram_parameter("iota", [P, P], BF16, isOutput=False)
    ones = nc.declare_dram_parameter("ones", [1, P], BF16, isOutput=False)
    ident = nc.declare_dram_parameter("ident", [P, P], BF16, isOutput=False)
    hTc3 = nc.declare_dram_parameter("hTc3", [D, NC * s.CPR], BF16, isOutput=False)
    invc = nc.declare_dram_parameter("invc", [s.SPC, 1], F32, isOutput=False)
    idxw = nc.declare_dram_parameter("idxw", [P, s.TOT // 16], I16, isOutput=False)
    dstw = nc.declare_dram_parameter("dstw", [P, s.TOTCH], BF16, isOutput=False)
    pidxw = nc.declare_dram_parameter("pidxw", [P, s.PTOT // 16], I16, isOutput=False)
    pdstw = nc.declare_dram_parameter("pdstw", [P, s.PTOTCH], BF16, isOutput=False)
    out_ext = nc.declare_dram_parameter("out", [s.SPC, D], F32, isOutput=True)

    n_tiles = s.NT

    with tile.TileContext(nc) as tc:
        with (
            tc.tile_pool(name="const", bufs=1) as cpool,
            tc.tile_pool(name="hT", bufs=2) as hpool,
            tc.tile_pool(name="idx", bufs=1) as ipool,
            tc.tile_pool(name="gath", bufs=26) as gpool,
            tc.tile_pool(name="hc3", bufs=2) as hc3pool,
            tc.tile_pool(name="oh", bufs=26) as opool,
            tc.tile_pool(name="sml", bufs=6) as spool,
            tc.tile_pool(name="zps", bufs=2, space="PSUM") as zpsum,
            tc.tile_pool(name="scps", bufs=ST, space="PSUM") as scpsum,
            tc.tile_pool(name="dram", bufs=1, space="DRAM") as dram,
        ):
            # ---- constants ----
            iota_t = cpool.tile([P, P], BF16)
            nc.sync.dma_start(out=iota_t[:], in_=iota[:, :])
            ones_t = cpool.tile([1, P], BF16)
            nc.sync.dma_start(out=ones_t[:], in_=ones[:, :])
            ident_t = cpool.tile([P, P], BF16)
            nc.sync.dma_start(out=ident_t[:], in_=ident[:, :])
            w_t = {}
            b_t = {}
            for li, (W, b) in enumerate(((W1, b1), (W2, b2))):
                for k in range(D // P):
                    wt = cpool.tile([P, D], BF16, tag=f"w{li}{k}", name=f"w{li}{k}")
                    w_t[(li, k)] = wt
                    nc.sync.dma_start(out=wt[:], in_=W[k * P:(k + 1) * P, :])
                bt = cpool.tile([1, D], BF16, tag=f"b{li}", name=f"b{li}")
                b_t[li] = bt
                nc.sync.dma_start(out=bt[:], in_=b[:, :])
            invc_t = cpool.tile([s.SPC, 1], F32)
            nc.sync.dma_start(out=invc_t[:], in_=invc[:, :])


            # ---- internal DRAM ----
            z_shard = dram.tile([s.NPC, D], BF16)
            z_full = [[dram.tile([s.CPR * NC, D], BF16, tag=f"zfull{li}_{k}",
                                 name=f"zfull{li}_{k}",
                                 addr_space="Local" if (li, k) == (0, 3)
                                 else "Shared")
                       for k in range(4)] for li in range(2)]
            h2_dram = dram.tile([s.NPC, D], BF16)
            pool_part = dram.tile([s.n_slots, D], F32)
            pool_red = dram.tile([s.SPC, D], F32)

            def tile_width(t):
                return min(P, s.NPC - t * P)

            swq_state = {"i": 0, "prev": None}

            def issue_gather(out_ap, in_ap, idx_ap, num):
                i = swq_state["i"]
                swq_state["i"] = i + 1
                gi = nc.gpsimd.dma_gather(
                    out_ap, in_ap, idx_ap, num, num, D,
                    queue_num=i % N_SWQ,
                )
                if swq_state["prev"] is not None:
                    add_dep_helper(gi.ins, swq_state["prev"], sync=False,
                                   reason="swdge lane/queue order")
                swq_state["prev"] = gi.ins
                return gi

            def z_tile(li, hT_tiles, t):
                """z = h @ W + b for one node tile -> z_shard rows."""
                tw = tile_width(t)
                zp = zpsum.tile([P, D], F32, tag="zp", space="PSUM",
                                name=f"zp{li}_{t}")
                for k in range(D // P):
                    nc.tensor.matmul(
                        out=zp[:tw],
                        lhsT=hT_tiles[k][:, t * P:t * P + tw],
                        rhs=w_t[(li, k)][:],
                        start=(k == 0),
                        stop=False,
                    )
                nc.tensor.matmul(
                    out=zp[:tw],
                    lhsT=ones_t[:1, :tw],
                    rhs=b_t[li][:],
                    start=False,
                    stop=True,
                )
                zt = spool.tile([P, D], BF16, tag="zt", name=f"zt{li}_{t}")
                nc.vector.tensor_copy(out=zt[:tw], in_=zp[:tw])
                nc.sync.dma_start(out=z_shard[t * P:t * P + tw, :],
                                  in_=zt[:tw])

            def z_phase(li, hT_tiles):
                for t in range(n_tiles):
                    z_tile(li, hT_tiles, t)

            def run_scatter(li, hT_next, call_hook=None):
                zf = z_full[li]
                psum_live = {}

                def flush_tile(t):
                    tw = tile_width(t)
                    ps = psum_live.pop(t)
                    ht = spool.tile([P, D], BF16, tag="ht")
                    nc.scalar.activation(
                        out=ht[:tw], in_=ps[:tw],
                        func=mybir.ActivationFunctionType.Tanh,
                    )
                    if li == 0:
                        # transpose h tile into the next layer's lhsT columns
                        for k in range(D // P):
                            tp = zpsum.tile([P, P], BF16, tag="zp",
                                            space="PSUM", name=f"tp{li}_{t}_{k}")
                            nc.tensor.transpose(
                                out=tp[:, :tw],
                                in_=ht[:tw, k * P:(k + 1) * P],
                                identity=ident_t[:tw, :tw],
                            )
                            nc.vector.tensor_copy(
                                out=hT_next[k][:, t * P:t * P + tw],
                                in_=tp[:, :tw])
                        z_tile(1, hT_next, t)
                    else:
                        nc.sync.dma_start(
                            out=h2_dram[t * P:t * P + tw, :], in_=ht[:tw])

                for call_i, (qq, c_off, n_ch) in enumerate(s.calls):
                    if call_hook is not None:
                        call_hook(call_i)
                    num = n_ch * P
                    g = gpool.tile([P, s.max_call_ch, D], BF16, tag="g")
                    issue_gather(
                        g[:, :n_ch, :],
                        zf[qq][:, :],
                        idx_t[:, c_off * 8:(c_off + n_ch) * 8],
                        num,
                    )
                    oh = opool.tile([P, s.max_call_ch, P], BF16, tag="oh")
                    nc.vector.tensor_tensor(
                        out=oh[:, :n_ch, :],
                        in0=iota_t[:, None, :].to_broadcast([P, n_ch, P]),
                        in1=dst_t[:, c_off:c_off + n_ch, None]
                            .to_broadcast([P, n_ch, P]),
                        op=mybir.AluOpType.is_equal,
                    )
                    for j in range(n_ch):
                        ci = c_off + j
                        t = int(s.chunk_tile[ci])
                        if s.first_chunk[t] == ci:
                            psum_live[t] = scpsum.tile(
                                [P, D], F32, tag="scp", space="PSUM", name=f"scp{t}")
                        nc.tensor.matmul(
                            out=psum_live[t][:],
                            lhsT=oh[:, j, :],
                            rhs=g[:, j, :],
                            start=(s.first_chunk[t] == ci),
                            stop=(s.last_chunk[t] == ci),
                        )
                        if s.last_chunk[t] == ci:
                            flush_tile(t)
                if call_hook is not None:
                    call_hook(len(s.calls))

            # ================= layer 1 =================
            idx_t = ipool.tile([P, s.TOT // 16], I16)
            nc.sync.dma_start(out=idx_t[:], in_=idxw[:, :])
            dst_t = ipool.tile([P, s.TOTCH], BF16)
            nc.sync.dma_start(out=dst_t[:], in_=dstw[:, :])
            pidx_t = ipool.tile([P, s.PTOT // 16], I16)
            nc.sync.dma_start(out=pidx_t[:], in_=pidxw[:, :])
            pdst_t = ipool.tile([P, s.PTOTCH], BF16)
            nc.sync.dma_start(out=pdst_t[:], in_=pdstw[:, :])
            hT_l1 = [hpool.tile([P, s.NPCP], BF16, tag="hT", name=f"hT1_{_k}") for _k in range(D // P)]
            for k in range(D // P):
                nc.sync.dma_start(out=hT_l1[k][:, :s.NPC],
                                  in_=hT0[k * P:(k + 1) * P, :])
            def ag_chunk(li, k):
                nc.gpsimd.collective_compute(
                    "AllGather", mybir.AluOpType.bypass,
                    replica_groups=[list(range(NC))],
                    ins=[z_shard[k * s.CPR:(k + 1) * s.CPR, :].opt()],
                    outs=[z_full[li][k].opt()],
                )

            z_phase(0, hT_l1)
            for k in range(3):
                ag_chunk(0, k)
            ag1_done = {0, 1, 2, 3}
            # chunk-3 z1 table computed locally on every core (PE is idle
            # during the AG chain; saves one serial mesh-AllGather)
            C3T = (NC * s.CPR) // P
            BL = 8
            for b in range(math.ceil(C3T / BL)):
                t0 = b * BL
                nt = min(BL, C3T - t0)
                cols = nt * P
                blks = []
                for k in range(D // P):
                    blk = hc3pool.tile([P, BL * P], BF16, tag=f"hc3{k}",
                                       name=f"hc3_{b}_{k}")
                    nc.sync.dma_start(
                        out=blk[:, :cols],
                        in_=hTc3[k * P:(k + 1) * P, t0 * P:t0 * P + cols])
                    blks.append(blk)
                for j in range(nt):
                    kt = t0 + j
                    zp = zpsum.tile([P, D], F32, tag="zp", space="PSUM",
                                    name=f"zc3p_{kt}")
                    for k in range(D // P):
                        nc.tensor.matmul(
                            out=zp[:],
                            lhsT=blks[k][:, j * P:(j + 1) * P],
                            rhs=w_t[(0, k)][:],
                            start=(k == 0),
                            stop=False,
                        )
                    nc.tensor.matmul(
                        out=zp[:], lhsT=ones_t[:1, :], rhs=b_t[0][:],
                        start=False, stop=True,
                    )
                    zt = spool.tile([P, D], BF16, tag="zt", name=f"ztc3_{kt}")
                    nc.vector.tensor_copy(out=zt[:], in_=zp[:])
                    nc.sync.dma_start(
                        out=z_full[0][3][kt * P:(kt + 1) * P, :], in_=zt[:])
            hT_l2 = [hpool.tile([P, s.NPCP], BF16, tag="hT", name=f"hT2_{_k}") for _k in range(D // P)]

            ag2_done = set()

            def call_hook(ci):
                for k in s.ag1_after_call.get(ci, []):
                    if k not in ag1_done:
                        ag1_done.add(k)
                        ag_chunk(0, k)
                for k in s.ag2_after_call.get(ci, []):
                    if k not in ag2_done:
                        ag2_done.add(k)
                        ag_chunk(1, k)

            run_scatter(0, hT_l2, call_hook=call_hook)
            for k in range(4):
                if k not in ag1_done:
                    ag_chunk(0, k)
                if k not in ag2_done:
                    ag_chunk(1, k)

            # ================= layer 2 =================
            run_scatter(1, None)

            # ================= pooling =================
            pfirst = np.full(s.PST, -1, dtype=np.int64)
            plast = np.full(s.PST, -1, dtype=np.int64)
            for ci, t in enumerate(s.pchunk_tile):
                if pfirst[t] < 0:
                    pfirst[t] = ci
                plast[t] = ci
            for sl in range(s.PST):
                tot_ch = int(s.pnch[sl])
                base_off = int(s.prun_off[sl])
                pp = scpsum.tile([P, D], F32, tag="scp", space="PSUM")
                done = 0
                while done < tot_ch:
                    n_ch = min(MAX_CALL_CH, tot_ch - done)
                    c_off = base_off + done
                    num = n_ch * P
                    g = gpool.tile([P, s.max_call_ch, D], BF16, tag="g")
                    issue_gather(
                        g[:, :n_ch, :],
                        h2_dram[:s.NPC, :],
                        pidx_t[:, c_off * 8:(c_off + n_ch) * 8],
                        num,
                    )
                    oh = opool.tile([P, s.max_call_ch, P], BF16, tag="oh")
                    nc.vector.tensor_tensor(
                        out=oh[:, :n_ch, :],
                        in0=iota_t[:, None, :].to_broadcast([P, n_ch, P]),
                        in1=pdst_t[:, c_off:c_off + n_ch, None]
                            .to_broadcast([P, n_ch, P]),
                        op=mybir.AluOpType.is_equal,
                    )
                    for j in range(n_ch):
                        nc.tensor.matmul(
                            out=pp[:],
                            lhsT=oh[:, j, :],
                            rhs=g[:, j, :],
                            start=(done + j == 0),
                            stop=(done + j == tot_ch - 1),
                        )
                    done += n_ch
                sw = min(P, s.n_slots - sl * P)
                pc = spool.tile([P, D], F32, tag="pc")
                nc.vector.tensor_copy(out=pc[:sw], in_=pp[:sw])
                nc.sync.dma_start(out=pool_part[sl * P:sl * P + sw, :],
                                  in_=pc[:sw])

            nc.gpsimd.collective_compute(
                "ReduceScatter", mybir.AluOpType.add,
                replica_groups=[list(range(NC))],
                ins=[pool_part.opt()], outs=[pool_red.opt()],
            )
            ot = spool.tile([s.SPC, D], F32, tag="ot")
            nc.sync.dma_start(out=ot[:], in_=pool_red[:, :])
            ot2 = spool.tile([s.SPC, D], F32, tag="ot2")
            nc.vector.tensor_scalar(
                out=ot2[:], in0=ot[:], scalar1=invc_t[:, :1], scalar2=None,
                op0=mybir.AluOpType.mult,
            )
            nc.sync.dma_start(out=out_ext[:, :], in_=ot2[:])

    nc.compile()
    return nc


_CACHE = {}


def _run(inputs, trace=False):
    h = np.asarray(inputs["h"], dtype=np.float32)
    W1 = np.asarray(inputs["W1"], dtype=np.float32)
    b1 = np.asarray(inputs["b1"], dtype=np.float32)
    W2 = np.asarray(inputs["W2"], dtype=np.float32)
    b2 = np.asarray(inputs["b2"], dtype=np.float32)
    edge_src = np.asarray(inputs["edge_src"])
    edge_dst = np.asarray(inputs["edge_dst"])
    seg_ids = np.asarray(inputs["seg_ids"])

    n_slots = 800
    s, in_maps = _preprocess(h, W1, b1, W2, b2, edge_src, edge_dst,
                             seg_ids, n_slots)
    key = (s.N, s.D, s.TOTCH, s.PTOTCH)
    if key not in _CACHE:
        _CACHE[key] = _build(s)
    nc = _CACHE[key]
    if trace:
        _install_ntff_hook()
    try:
        res = run_bass_kernel_spmd(nc, in_maps, core_ids=list(range(NC)),
                                   trace=trace)
    except Exception:
        _axon_reset()
        res = run_bass_kernel_spmd(nc, in_maps, core_ids=list(range(NC)),
                                   trace=trace)
    parts = [res.results[c]["out"] for c in range(NC)]
    full = np.concatenate(parts, axis=0).astype(np.float32)
    return full, res


def kernel(**inputs) -> np.ndarray:
    full, _ = _run(inputs, trace=False)
    return full.reshape(16, 50, full.shape[1])


# revision 28
# speedup vs baseline: 1.1750x; 1.1750x over previous
"""Distributed Trainium2 kernel for a 2-layer GCN + ragged segment-mean pooling.

reference math:
    z1 = h @ W1 + b1;  h1 = tanh(segment_sum(z1[edge_src], edge_dst, N))
    z2 = h1 @ W2 + b2; h2 = tanh(segment_sum(z2[edge_src], edge_dst, N))
    out[s] = mean over nodes with seg_ids==s of h2  -> [B, MC, H]

Sharding: nodes split contiguously over 8 cores; weights replicated.
Per layer: local z matmul (bf16, PE) -> AllGather(z) -> per-core edge
gather (dma_gather, dst-sorted edges in 128-chunks, int16 idx against
4 quarter bases) -> one-hot matmul scatter-add into PSUM -> tanh.
Pooling: seg-sorted node gather + one-hot matmul into 800 slots,
ReduceScatter(add), scale by host-precomputed 1/count.
"""
import math
import sys
import types

import numpy as np

sys.path.insert(0, "/opt/trn_rl_repo")
if "/root/.axon_site" not in sys.path:
    sys.path.insert(0, "/root/.axon_site")

import concourse.bass as bass
import concourse.bacc as bacc
import concourse.tile as tile
from concourse.tile import add_dep_helper
import concourse.mybir as mybir
from concourse.bass_utils import run_bass_kernel_spmd

BF16 = mybir.dt.float16  # compute dtype (fp16: 10-bit mantissa, values O(1))
F32 = mybir.dt.float32
I16 = mybir.dt.int16
bf16_np = mybir.dt.np(BF16)

NC = 8          # cores
P = 128         # partitions / tile width
QB = 32768      # int16-index quarter size
ST = 3          # node tiles per supertile (2 supertiles in flight = 6 PSUM banks)
MAX_CALL_CH = 4  # chunks per dma_gather call (512 idxs)
N_SWQ = 4       # SWDGE queues to rotate over


def _install_ntff_hook():
    try:
        import antenv
        if getattr(antenv, "axon_hooks", None) is not None:
            return
        mod = types.ModuleType("antenv.axon_hooks")
        _hook = [None]
        mod.set_axon_ntff_profile_hook = lambda h: _hook.__setitem__(0, h)
        mod.get_axon_ntff_profile_hook = lambda: _hook[0]
        sys.modules["antenv.axon_hooks"] = mod
        antenv.axon_hooks = mod
        from trn_agent_boot.trn_boot import _ntff_profile_via_ctypes
        mod.set_axon_ntff_profile_hook(
            _ntff_profile_via_ctypes("/opt/axon/libaxon_pjrt.so")
        )
    except Exception:
        pass


def _axon_reset():
    try:
        import ctypes
        import time
        lib = ctypes.CDLL("/opt/axon/libaxon_pjrt.so")
        lib.axon_reset.restype = ctypes.c_int64
        lib.axon_reset()
        time.sleep(3)
    except Exception:
        pass


def _wrap_idx(stream_i16: np.ndarray) -> np.ndarray:
    """[TOT] int16 -> [128, TOT//16] wrapped in 16 partitions, replicated x8."""
    w = stream_i16.reshape(-1, 16).T  # [16, TOT/16]
    return np.tile(w, (8, 1)).astype(np.int16)


class _Sched:
    """Static (core-independent) schedule shared by the SPMD graph."""


def _placement(N, src, dst):
    """Assign nodes to (core, tile, slot) positions: balance in-degree per
    tile and out-degree mass per quarter group. Returns (pos[node], NT)."""
    E = len(src)
    npcn = N // NC
    base = max(math.ceil(npcn / P), int(round(E / (NC * 4 * 463.0))))
    NT = 4 * math.ceil(base / 4)
    T_ALL = NC * NT
    indeg = np.bincount(dst, minlength=N)
    order = np.argsort(-indeg, kind="stable")
    # snake round-robin over all tiles
    rounds = math.ceil(N / T_ALL)
    fwd = np.arange(T_ALL)
    tile_seq = np.concatenate(
        [fwd if r % 2 == 0 else fwd[::-1] for r in range(rounds)])[:N]
    gtile = np.empty(N, dtype=np.int64)   # global tile per node
    gtile[order] = tile_seq
    slot = np.empty(N, dtype=np.int64)
    srt = np.argsort(gtile, kind="stable")
    starts = np.searchsorted(gtile[srt], np.arange(T_ALL))
    slot[srt] = np.arange(N) - starts[gtile[srt]]
    assert slot.max() < P
    # quarter grouping per core: snake tiles by out-degree mass into 4 groups
    outdeg = np.bincount(src, minlength=N).astype(np.int64)
    tile_mass = np.bincount(gtile, weights=outdeg, minlength=T_ALL)
    new_tile = np.empty(T_ALL, dtype=np.int64)
    GQ = NT // 4
    for c in range(NC):
        tl = np.arange(c * NT, (c + 1) * NT)
        morder = np.argsort(-tile_mass[tl], kind="stable")
        grp_fill = np.zeros(4, dtype=np.int64)
        for i, ti in enumerate(morder):
            g = i % 8
            g = g if g < 4 else 7 - g
            new_tile[tl[ti]] = c * NT + g * GQ + grp_fill[g]
            grp_fill[g] += 1
    gtile2 = new_tile[gtile]
    pos = gtile2 * P + slot
    return pos, NT


def _preprocess(h, W1, b1, W2, b2, edge_src, edge_dst, seg_ids, n_slots):
    N, D = h.shape
    assert N % NC == 0
    src0 = edge_src.astype(np.int64)
    dst0 = edge_dst.astype(np.int64)
    seg0 = seg_ids.astype(np.int64)

    npos, NT = _placement(N, src0, dst0)
    NPC = NT * P              # position slots per core
    NQ = 4
    CPR = NPC // 4            # rows per rank per AG chunk
    NPCP = NPC
    SPC = n_slots // NC
    PST = math.ceil(n_slots / P)     # pool slot tiles

    src = npos[src0]
    dst = npos[dst0]

    # ---- edge schedule: runs keyed by (core, tile, quarter) ----
    core = dst // NPC
    t_loc = (dst - core * NPC) // P
    src_rank = src // NPC
    src_loc = src - src_rank * NPC
    q = src_loc // CPR
    key = (core * NT + t_loc) * NQ + q
    order = np.argsort(key, kind="stable")
    skey = key[order]
    ssrc = src[order]
    sdst = dst[order]
    counts = np.bincount(key, minlength=NC * NT * NQ).reshape(NC, NT, NQ)
    nch = (counts.max(axis=0) + P - 1) // P          # [NT, NQ] chunks
    empty = nch.sum(axis=1) == 0
    nch[empty, 0] = 1                                 # every tile gets >=1 chunk

    # stream order: wave emission — st[q0..q2], then (st-1)[q3] deferred
    NST = math.ceil(NT / ST)
    run_order = []
    for st in range(NST):
        for qq in range(min(NQ, 3)):
            run_order.append((st, qq))
        if st >= 1 and NQ == 4:
            run_order.append((st - 1, 3))
    if NQ == 4:
        run_order.append((NST - 1, 3))
    calls = []       # (q, chunk_off, n_chunks) per gather call
    chunk_tile = []  # owning node-tile per chunk, stream order
    chunk_seq = [[[] for _ in range(NQ)] for _ in range(NT)]  # (t,q) -> stream chunk ids
    run_call_end = {}  # (st,q) -> index past the run's last call
    off = 0
    for (st, qq) in run_order:
        tiles = range(st * ST, min((st + 1) * ST, NT))
        call_off = off
        tl = list(tiles)
        for p0 in range(0, len(tl), 2):
            pair = tl[p0:p0 + 2]
            mx = max((int(nch[t, qq]) for t in pair), default=0)
            for k in range(mx):
                for t in pair:
                    if k < nch[t, qq]:
                        chunk_seq[t][qq].append(off)
                        chunk_tile.append(t)
                        off += 1
        co = call_off
        while co < off:
            n = min(MAX_CALL_CH, off - co)
            calls.append((qq, co, n))
            co += n
        run_call_end[(st, qq)] = len(calls)
    TOTCH = off
    # AG2 chunk k triggers after the run that flushes its last z2 tile
    # (q3 of the covering supertile), +2 calls of slack
    ag2_after_call = {}
    if NQ == 4:
        for k in range(4):
            t_last = (min((k + 1) * CPR, NPC) - 1) // P
            st_k = min(t_last // ST, NST - 1)
            ci = min(run_call_end[(st_k, 3)] + 2, len(calls))
            ag2_after_call.setdefault(ci, []).append(k)
    else:
        ag2_after_call = {}
    # AG1 chunk k (k>=1) triggers just before its first consumer run
    ag1_after_call = {}
    if NQ == 4:
        ag1_after_call.setdefault(run_call_end[(0, 0)], []).append(1)
        ag1_after_call.setdefault(run_call_end[(0, 1)], []).append(2)
        ag1_after_call.setdefault(run_call_end[(1, 2)], []).append(3)
    TOT = TOTCH * P
    chunk_tile = np.asarray(chunk_tile)
    # start/stop chunk per tile
    first_chunk = np.full(NT, -1, dtype=np.int64)
    last_chunk = np.full(NT, -1, dtype=np.int64)
    for ci, t in enumerate(chunk_tile):
        if first_chunk[t] < 0:
            first_chunk[t] = ci
        last_chunk[t] = ci

    # ---- per-core padded streams ----
    # chunk-id table [NT, NQ, max_nch] -> stream chunk id
    max_nch = int(nch.max())
    chunk_id_tab = np.zeros((NT, NQ, max_nch), dtype=np.int64)
    for t in range(NT):
        for qq in range(NQ):
            for k, cid in enumerate(chunk_seq[t][qq]):
                chunk_id_tab[t, qq, k] = cid
    group_start = np.searchsorted(skey, np.arange(NC * NT * NQ), side="left")
    rank_in_run = np.arange(len(skey)) - group_start[skey]
    e_t = (sdst - (sdst // NPC) * NPC) // P
    e_rank = ssrc // NPC
    e_loc = ssrc - e_rank * NPC
    e_q = e_loc // CPR
    e_core = sdst // NPC
    pos = chunk_id_tab[e_t, e_q, rank_in_run // P] * P + rank_in_run % P

    idx_stream = np.zeros((NC, TOT), dtype=np.int16)
    dst_stream = np.full((NC, TOT), -1.0, dtype=np.float32)
    idx_stream[e_core, pos] = (e_rank * CPR + e_loc - e_q * CPR).astype(np.int16)
    dst_stream[e_core, pos] = (sdst - e_core * NPC - e_t * P).astype(np.float32)

    # ---- pooling schedule: runs keyed by (core, slot_tile) ----
    seg = seg0
    ncore = npos // NPC
    stile = seg // P
    pkey = ncore * PST + stile
    porder = np.argsort(pkey, kind="stable")
    pskey = pkey[porder]
    pseg = seg[porder]
    pnode_loc = (npos - ncore * NPC)[porder]
    pcounts = np.bincount(pkey, minlength=NC * PST).reshape(NC, PST)
    pnch = (pcounts.max(axis=0) + P - 1) // P
    pnch[pnch == 0] = 1
    prun_off = np.zeros(PST, dtype=np.int64)
    poff = 0
    pchunk_tile = []
    for s in range(PST):
        prun_off[s] = poff
        pchunk_tile.extend([s] * int(pnch[s]))
        poff += int(pnch[s])
    PTOTCH = poff
    PTOT = PTOTCH * P
    pchunk_tile = np.asarray(pchunk_tile)

    pgroup_start = np.searchsorted(pskey, np.arange(NC * PST), side="left")
    prank = np.arange(len(pskey)) - pgroup_start[pskey]
    p_core = pskey // PST
    p_s = pskey % PST
    ppos = prun_off[p_s] * P + prank

    pidx_stream = np.zeros((NC, PTOT), dtype=np.int16)
    pdst_stream = np.full((NC, PTOT), -1.0, dtype=np.float32)
    pidx_stream[p_core, ppos] = pnode_loc.astype(np.int16)
    pdst_stream[p_core, ppos] = (pseg - p_s * P).astype(np.float32)

    cnts = np.bincount(seg, minlength=n_slots).astype(np.float32)
    inv = 1.0 / np.maximum(cnts, 1.0)

    # ---- host-side tensors per core ----
    h_pos = np.zeros((NC * NPC, D), dtype=np.float32)
    h_pos[npos] = h
    hbf = h_pos.astype(bf16_np)
    iota = np.tile(np.arange(P, dtype=np.float32), (P, 1)).astype(bf16_np)
    ones = np.ones((1, P), dtype=np.float32).astype(bf16_np)
    ident = np.eye(P, dtype=np.float32).astype(bf16_np)
    in_maps = []
    for c in range(NC):
        in_maps.append({
            "hT0": np.ascontiguousarray(hbf[c * NPC:(c + 1) * NPC].T),
            "W1": W1.astype(bf16_np),
            "W2": W2.astype(bf16_np),
            "b1": b1.reshape(1, D).astype(bf16_np),
            "b2": b2.reshape(1, D).astype(bf16_np),
            "iota": iota,
            "ones": ones,
            "ident": ident,
            "invc": inv[c * SPC:(c + 1) * SPC].reshape(SPC, 1),
            "idxw": _wrap_idx(idx_stream[c]),
            "dstw": np.ascontiguousarray(
                dst_stream[c].reshape(TOTCH, P).T.astype(bf16_np)),
            "pidxw": _wrap_idx(pidx_stream[c]),
            "pdstw": np.ascontiguousarray(
                pdst_stream[c].reshape(PTOTCH, P).T.astype(bf16_np)),
        })

    s = _Sched()
    s.N, s.D, s.NPC, s.NT, s.NQ, s.NPCP = N, D, NPC, NT, NQ, NPCP
    s.n_slots, s.SPC, s.PST = n_slots, SPC, PST
    s.NST = NST
    s.nch, s.calls, s.chunk_tile = nch, calls, chunk_tile
    s.ag2_after_call = ag2_after_call if NQ == 4 else {}
    s.ag1_after_call = ag1_after_call if NQ == 4 else {}
    s.first_chunk, s.last_chunk = first_chunk, last_chunk
    s.TOTCH, s.TOT = TOTCH, TOT
    s.pnch, s.pchunk_tile, s.PTOTCH, s.PTOT = pnch, pchunk_tile, PTOTCH, PTOT
    s.prun_off = prun_off
    s.max_call_ch = max(n for (_, _, n) in calls)
    s.CPR = CPR
    return s, in_maps


def _build(s):
    D = s.D
    nc = bacc.Bacc("TRN2", target_bir_lowering=False, debug=False,
                   num_swdge_queues=N_SWQ)

    hT0 = nc.declare_dram_parameter("hT0", [D, s.NPC], BF16, isOutput=False)
    W1 = nc.declare_dram_parameter("W1", [D, D], BF16, isOutput=False)
    W2 = nc.declare_dram_parameter("W2", [D, D], BF16, isOutput=False)
    b1 = nc.declare_dram_parameter("b1", [1, D], BF16, isOutput=False)
    b2 = nc.declare_dram_parameter("b2", [1, D], BF16, isOutput=False)
    iota = nc.declare_dram_parameter("iota", [P, P], BF16, isOutput=False)
    ones = nc.declare_dram_parameter("ones", [1, P], BF16, isOutput=False)
    ident = nc.declare_dram_parameter("ident", [P, P], BF16, isOutput=False)
    invc = nc.declare_dram_parameter("invc", [s.SPC, 1], F32, isOutput=False)
    idxw = nc.declare_dram_parameter("idxw", [P, s.TOT // 16], I16, isOutput=False)
    dstw = nc.declare_dram_parameter("dstw", [P, s.TOTCH], BF16, isOutput=False)
    pidxw = nc.declare_dram_parameter("pidxw", [P, s.PTOT // 16], I16, isOutput=False)
    pdstw = nc.declare_dram_parameter("pdstw", [P, s.PTOTCH], BF16, isOutput=False)
    out_ext = nc.declare_dram_parameter("out", [s.SPC, D], F32, isOutput=True)

    n_tiles = s.NT

    with tile.TileContext(nc) as tc:
        with (
            tc.tile_pool(name="const", bufs=1) as cpool,
            tc.tile_pool(name="hT", bufs=2) as hpool,
            tc.tile_pool(name="idx", bufs=1) as ipool,
            tc.tile_pool(name="gath", bufs=26) as gpool,
            tc.tile_pool(name="oh", bufs=26) as opool,
            tc.tile_pool(name="sml", bufs=6) as spool,
            tc.tile_pool(name="zps", bufs=2, space="PSUM") as zpsum,
            tc.tile_pool(name="scps", bufs=ST, space="PSUM") as scpsum,
            tc.tile_pool(name="dram", bufs=1, space="DRAM") as dram,
        ):
            # ---- constants ----
            iota_t = cpool.tile([P, P], BF16)
            nc.sync.dma_start(out=iota_t[:], in_=iota[:, :])
            ones_t = cpool.tile([1, P], BF16)
            nc.sync.dma_start(out=ones_t[:], in_=ones[:, :])
            ident_t = cpool.tile([P, P], BF16)
            nc.sync.dma_start(out=ident_t[:], in_=ident[:, :])
            w_t = {}
            b_t = {}
            for li, (W, b) in enumerate(((W1, b1), (W2, b2))):
                for k in range(D // P):
                    wt = cpool.tile([P, D], BF16, tag=f"w{li}{k}", name=f"w{li}{k}")
                    w_t[(li, k)] = wt
                    nc.sync.dma_start(out=wt[:], in_=W[k * P:(k + 1) * P, :])
                bt = cpool.tile([1, D], BF16, tag=f"b{li}", name=f"b{li}")
                b_t[li] = bt
                nc.sync.dma_start(out=bt[:], in_=b[:, :])
            invc_t = cpool.tile([s.SPC, 1], F32)
            nc.sync.dma_start(out=invc_t[:], in_=invc[:, :])


            # ---- internal DRAM ----
            z_shard = dram.tile([s.NPC, D], BF16)
            z_full = [[dram.tile([s.CPR * NC, D], BF16, tag=f"zfull{li}_{k}",
                                 name=f"zfull{li}_{k}", addr_space="Shared")
                       for k in range(4)] for li in range(2)]
            h2_dram = dram.tile([s.NPC, D], BF16)
            pool_part = dram.tile([s.n_slots, D], F32)
            pool_red = dram.tile([s.SPC, D], F32)

            def tile_width(t):
                return min(P, s.NPC - t * P)

            swq_state = {"i": 0, "prev": None}

            def issue_gather(out_ap, in_ap, idx_ap, num):
                i = swq_state["i"]
                swq_state["i"] = i + 1
                gi = nc.gpsimd.dma_gather(
                    out_ap, in_ap, idx_ap, num, num, D,
                    queue_num=i % N_SWQ,
                )
                if swq_state["prev"] is not None:
                    add_dep_helper(gi.ins, swq_state["prev"], sync=False,
                                   reason="swdge lane/queue order")
                swq_state["prev"] = gi.ins
                return gi

            def z_tile(li, hT_tiles, t):
                """z = h @ W + b for one node tile -> z_shard rows."""
                tw = tile_width(t)
                zp = zpsum.tile([P, D], F32, tag="zp", space="PSUM",
                                name=f"zp{li}_{t}")
                for k in range(D // P):
                    nc.tensor.matmul(
                        out=zp[:tw],
                        lhsT=hT_tiles[k][:, t * P:t * P + tw],
                        rhs=w_t[(li, k)][:],
                        start=(k == 0),
                        stop=False,
                    )
                nc.tensor.matmul(
                    out=zp[:tw],
                    lhsT=ones_t[:1, :tw],
                    rhs=b_t[li][:],
                    start=False,
                    stop=True,
                )
                zt = spool.tile([P, D], BF16, tag="zt", name=f"zt{li}_{t}")
                nc.vector.tensor_copy(out=zt[:tw], in_=zp[:tw])
                nc.sync.dma_start(out=z_shard[t * P:t * P + tw, :],
                                  in_=zt[:tw])

            def z_phase(li, hT_tiles):
                for t in range(n_tiles):
                    z_tile(li, hT_tiles, t)

            def run_scatter(li, hT_next, call_hook=None):
                zf = z_full[li]
                psum_live = {}

                def flush_tile(t):
                    tw = tile_width(t)
                    ps = psum_live.pop(t)
                    ht = spool.tile([P, D], BF16, tag="ht")
                    nc.scalar.activation(
                        out=ht[:tw], in_=ps[:tw],
                        func=mybir.ActivationFunctionType.Tanh,
                    )
                    if li == 0:
                        # transpose h tile into the next layer's lhsT columns
                        for k in range(D // P):
                            tp = zpsum.tile([P, P], BF16, tag="zp",
                                            space="PSUM", name=f"tp{li}_{t}_{k}")
                            nc.tensor.transpose(
                                out=tp[:, :tw],
                                in_=ht[:tw, k * P:(k + 1) * P],
                                identity=ident_t[:tw, :tw],
                            )
                            nc.vector.tensor_copy(
                                out=hT_next[k][:, t * P:t * P + tw],
                                in_=tp[:, :tw])
                        z_tile(1, hT_next, t)
                    else:
                        nc.sync.dma_start(
                            out=h2_dram[t * P:t * P + tw, :], in_=ht[:tw])

                for call_i, (qq, c_off, n_ch) in enumerate(s.calls):
                    if call_hook is not None:
                        call_hook(call_i)
                    num = n_ch * P
                    g = gpool.tile([P, s.max_call_ch, D], BF16, tag="g")
                    issue_gather(
                        g[:, :n_ch, :],
                        zf[qq][:, :],
                        idx_t[:, c_off * 8:(c_off + n_ch) * 8],
                        num,
                    )
                    oh = opool.tile([P, s.max_call_ch, P], BF16, tag="oh")
                    nc.vector.tensor_tensor(
                        out=oh[:, :n_ch, :],
                        in0=iota_t[:, None, :].to_broadcast([P, n_ch, P]),
                        in1=dst_t[:, c_off:c_off + n_ch, None]
                            .to_broadcast([P, n_ch, P]),
                        op=mybir.AluOpType.is_equal,
                    )
                    for j in range(n_ch):
                        ci = c_off + j
                        t = int(s.chunk_tile[ci])
                        if s.first_chunk[t] == ci:
                            psum_live[t] = scpsum.tile(
                                [P, D], F32, tag="scp", space="PSUM", name=f"scp{t}")
                        nc.tensor.matmul(
                            out=psum_live[t][:],
                            lhsT=oh[:, j, :],
                            rhs=g[:, j, :],
                            start=(s.first_chunk[t] == ci),
                            stop=(s.last_chunk[t] == ci),
                        )
                        if s.last_chunk[t] == ci:
                            flush_tile(t)
                if call_hook is not None:
                    call_hook(len(s.calls))

            # ================= layer 1 =================
            idx_t = ipool.tile([P, s.TOT // 16], I16)
            nc.sync.dma_start(out=idx_t[:], in_=idxw[:, :])
            dst_t = ipool.tile([P, s.TOTCH], BF16)
            nc.sync.dma_start(out=dst_t[:], in_=dstw[:, :])
            pidx_t = ipool.tile([P, s.PTOT // 16], I16)
            nc.sync.dma_start(out=pidx_t[:], in_=pidxw[:, :])
            pdst_t = ipool.tile([P, s.PTOTCH], BF16)
            nc.sync.dma_start(out=pdst_t[:], in_=pdstw[:, :])
            hT_l1 = [hpool.tile([P, s.NPCP], BF16, tag="hT", name=f"hT1_{_k}") for _k in range(D // P)]
            for k in range(D // P):
                nc.sync.dma_start(out=hT_l1[k][:, :s.NPC],
                                  in_=hT0[k * P:(k + 1) * P, :])
            def ag_chunk(li, k):
                nc.gpsimd.collective_compute(
                    "AllGather", mybir.AluOpType.bypass,
                    replica_groups=[list(range(NC))],
                    ins=[z_shard[k * s.CPR:(k + 1) * s.CPR, :].opt()],
                    outs=[z_full[li][k].opt()],
                )

            z_phase(0, hT_l1)
            for k in range(4):
                ag_chunk(0, k)
            ag1_done = {0, 1, 2, 3}
            hT_l2 = [hpool.tile([P, s.NPCP], BF16, tag="hT", name=f"hT2_{_k}") for _k in range(D // P)]

            ag2_done = set()

            def call_hook(ci):
                for k in s.ag1_after_call.get(ci, []):
                    if k not in ag1_done:
                        ag1_done.add(k)
                        ag_chunk(0, k)
                for k in s.ag2_after_call.get(ci, []):
                    if k not in ag2_done:
                        ag2_done.add(k)
                        ag_chunk(1, k)

            run_scatter(0, hT_l2, call_hook=call_hook)
            for k in range(4):
                if k not in ag1_done:
                    ag_chunk(0, k)
                if k not in ag2_done:
                    ag_chunk(1, k)

            # ================= layer 2 =================
            run_scatter(1, None)

            # ================= pooling =================
            pfirst = np.full(s.PST, -1, dtype=np.int64)
            plast = np.full(s.PST, -1, dtype=np.int64)
            for ci, t in enumerate(s.pchunk_tile):
                if pfirst[t] < 0:
                    pfirst[t] = ci
                plast[t] = ci
            for sl in range(s.PST):
                tot_ch = int(s.pnch[sl])
                base_off = int(s.prun_off[sl])
                pp = scpsum.tile([P, D], F32, tag="scp", space="PSUM")
                done = 0
                while done < tot_ch:
                    n_ch = min(MAX_CALL_CH, tot_ch - done)
                    c_off = base_off + done
                    num = n_ch * P
                    g = gpool.tile([P, s.max_call_ch, D], BF16, tag="g")
                    issue_gather(
                        g[:, :n_ch, :],
                        h2_dram[:s.NPC, :],
                        pidx_t[:, c_off * 8:(c_off + n_ch) * 8],
                        num,
                    )
                    oh = opool.tile([P, s.max_call_ch, P], BF16, tag="oh")
                    nc.vector.tensor_tensor(
                        out=oh[:, :n_ch, :],
                        in0=iota_t[:, None, :].to_broadcast([P, n_ch, P]),
                        in1=pdst_t[:, c_off:c_off + n_ch, None]
                            .to_broadcast([P, n_ch, P]),
                        op=mybir.AluOpType.is_equal,
                    )
                    for j in range(n_ch):
                        nc.tensor.matmul(
                            out=pp[:],
                            lhsT=oh[:, j, :],
                            rhs=g[:, j, :],
                            start=(done + j == 0),
                            stop=(done + j == tot_ch - 1),
                        )
                    done += n_ch
                sw = min(P, s.n_slots - sl * P)
                pc = spool.tile([P, D], F32, tag="pc")
                nc.vector.tensor_copy(out=pc[:sw], in_=pp[:sw])
                nc.sync.dma_start(out=pool_part[sl * P:sl * P + sw, :],
                                  in_=pc[:sw])

            nc.gpsimd.collective_compute(
                "ReduceScatter", mybir.AluOpType.add,
                replica_groups=[list(range(NC))],
                ins=[pool_part.opt()], outs=[pool_red.opt()],
            )
            ot = spool.tile([s.SPC, D], F32, tag="ot")
            nc.sync.dma_start(out=ot[:], in_=pool_red[:, :])
            ot2 = spool.tile([s.SPC, D], F32, tag="ot2")
            nc.vector.tensor_scalar(
                out=ot2[:], in0=ot[:], scalar1=invc_t[:, :1], scalar2=None,
                op0=mybir.AluOpType.mult,
            )
            nc.sync.dma_start(out=out_ext[:, :], in_=ot2[:])

    nc.compile()
    return nc


_CACHE = {}


def _run(inputs, trace=False):
    h = np.asarray(inputs["h"], dtype=np.float32)
    W1 = np.asarray(inputs["W1"], dtype=np.float32)
    b1 = np.asarray(inputs["b1"], dtype=np.float32)
    W2 = np.asarray(inputs["W2"], dtype=np.float32)
    b2 = np.asarray(inputs["b2"], dtype=np.float32)
    edge_src = np.asarray(inputs["edge_src"])
    edge_dst = np.asarray(inputs["edge_dst"])
    seg_ids = np.asarray(inputs["seg_ids"])

    n_slots = 800
    s, in_maps = _preprocess(h, W1, b1, W2, b2, edge_src, edge_dst,
                             seg_ids, n_slots)
    key = (s.N, s.D, s.TOTCH, s.PTOTCH)
    if key not in _CACHE:
        _CACHE[key] = _build(s)
    nc = _CACHE[key]
    if trace:
        _install_ntff_hook()
    try:
        res = run_bass_kernel_spmd(nc, in_maps, core_ids=list(range(NC)),
                                   trace=trace)
    except Exception:
        _axon_reset()
        res = run_bass_kernel_spmd(nc, in_maps, core_ids=list(range(NC)),
                                   trace=trace)
    parts = [res.results[c]["out"] for c in range(NC)]
    full = np.concatenate(parts, axis=0).astype(np.float32)
    return full, res


def kernel(**inputs) -> np.ndarray:
    full, _ = _run(inputs, trace=False)
    return full.reshape(16, 50, full.shape[1])


# revision 29
# speedup vs baseline: 1.1859x; 1.0093x over previous
"""Distributed Trainium2 kernel for a 2-layer GCN + ragged segment-mean pooling.

reference math:
    z1 = h @ W1 + b1;  h1 = tanh(segment_sum(z1[edge_src], edge_dst, N))
    z2 = h1 @ W2 + b2; h2 = tanh(segment_sum(z2[edge_src], edge_dst, N))
    out[s] = mean over nodes with seg_ids==s of h2  -> [B, MC, H]

Sharding: nodes split contiguously over 8 cores; weights replicated.
Per layer: local z matmul (bf16, PE) -> AllGather(z) -> per-core edge
gather (dma_gather, dst-sorted edges in 128-chunks, int16 idx against
4 quarter bases) -> one-hot matmul scatter-add into PSUM -> tanh.
Pooling: seg-sorted node gather + one-hot matmul into 800 slots,
ReduceScatter(add), scale by host-precomputed 1/count.
"""
import math
import sys
import types

import numpy as np

sys.path.insert(0, "/opt/trn_rl_repo")
if "/root/.axon_site" not in sys.path:
    sys.path.insert(0, "/root/.axon_site")

import concourse.bass as bass
import concourse.bacc as bacc
import concourse.tile as tile
from concourse.tile import add_dep_helper
import concourse.mybir as mybir
from concourse.bass_utils import run_bass_kernel_spmd

BF16 = mybir.dt.float16  # compute dtype (fp16: 10-bit mantissa, values O(1))
F32 = mybir.dt.float32
I16 = mybir.dt.int16
bf16_np = mybir.dt.np(BF16)

NC = 8          # cores
P = 128         # partitions / tile width
QB = 32768      # int16-index quarter size
ST = 3          # node tiles per supertile (2 supertiles in flight = 6 PSUM banks)
MAX_CALL_CH = 4  # chunks per dma_gather call (512 idxs)
N_SWQ = 4       # SWDGE queues to rotate over


def _install_ntff_hook():
    try:
        import antenv
        if getattr(antenv, "axon_hooks", None) is not None:
            return
        mod = types.ModuleType("antenv.axon_hooks")
        _hook = [None]
        mod.set_axon_ntff_profile_hook = lambda h: _hook.__setitem__(0, h)
        mod.get_axon_ntff_profile_hook = lambda: _hook[0]
        sys.modules["antenv.axon_hooks"] = mod
        antenv.axon_hooks = mod
        from trn_agent_boot.trn_boot import _ntff_profile_via_ctypes
        mod.set_axon_ntff_profile_hook(
            _ntff_profile_via_ctypes("/opt/axon/libaxon_pjrt.so")
        )
    except Exception:
        pass


def _axon_reset():
    try:
        import ctypes
        import time
        lib = ctypes.CDLL("/opt/axon/libaxon_pjrt.so")
        lib.axon_reset.restype = ctypes.c_int64
        lib.axon_reset()
        time.sleep(3)
    except Exception:
        pass


def _wrap_idx(stream_i16: np.ndarray) -> np.ndarray:
    """[TOT] int16 -> [128, TOT//16] wrapped in 16 partitions, replicated x8."""
    w = stream_i16.reshape(-1, 16).T  # [16, TOT/16]
    return np.tile(w, (8, 1)).astype(np.int16)


class _Sched:
    """Static (core-independent) schedule shared by the SPMD graph."""


def _placement(N, src, dst):
    """Assign nodes to (core, tile, slot) positions: balance in-degree per
    tile and out-degree mass per quarter group. Returns (pos[node], NT)."""
    E = len(src)
    npcn = N // NC
    base = max(math.ceil(npcn / P), int(round(E / (NC * 4 * 463.0))))
    NT = 4 * math.ceil(base / 4)
    T_ALL = NC * NT
    indeg = np.bincount(dst, minlength=N)
    order = np.argsort(-indeg, kind="stable")
    # snake round-robin over all tiles
    rounds = math.ceil(N / T_ALL)
    fwd = np.arange(T_ALL)
    tile_seq = np.concatenate(
        [fwd if r % 2 == 0 else fwd[::-1] for r in range(rounds)])[:N]
    gtile = np.empty(N, dtype=np.int64)   # global tile per node
    gtile[order] = tile_seq
    slot = np.empty(N, dtype=np.int64)
    srt = np.argsort(gtile, kind="stable")
    starts = np.searchsorted(gtile[srt], np.arange(T_ALL))
    slot[srt] = np.arange(N) - starts[gtile[srt]]
    assert slot.max() < P
    # quarter grouping per core: snake tiles by out-degree mass into 4 groups
    outdeg = np.bincount(src, minlength=N).astype(np.int64)
    tile_mass = np.bincount(gtile, weights=outdeg, minlength=T_ALL)
    new_tile = np.empty(T_ALL, dtype=np.int64)
    GQ = NT // 4
    for c in range(NC):
        tl = np.arange(c * NT, (c + 1) * NT)
        morder = np.argsort(-tile_mass[tl], kind="stable")
        grp_fill = np.zeros(4, dtype=np.int64)
        for i, ti in enumerate(morder):
            g = i % 8
            g = g if g < 4 else 7 - g
            new_tile[tl[ti]] = c * NT + g * GQ + grp_fill[g]
            grp_fill[g] += 1
    gtile2 = new_tile[gtile]
    pos = gtile2 * P + slot
    return pos, NT


def _preprocess(h, W1, b1, W2, b2, edge_src, edge_dst, seg_ids, n_slots):
    N, D = h.shape
    assert N % NC == 0
    src0 = edge_src.astype(np.int64)
    dst0 = edge_dst.astype(np.int64)
    seg0 = seg_ids.astype(np.int64)

    npos, NT = _placement(N, src0, dst0)
    NPC = NT * P              # position slots per core
    NQ = 4
    CPR = NPC // 4            # rows per rank per AG chunk
    NPCP = NPC
    SPC = n_slots // NC
    PST = math.ceil(n_slots / P)     # pool slot tiles

    src = npos[src0]
    dst = npos[dst0]

    # ---- edge schedule: runs keyed by (core, tile, quarter) ----
    core = dst // NPC
    t_loc = (dst - core * NPC) // P
    src_rank = src // NPC
    src_loc = src - src_rank * NPC
    q = src_loc // CPR
    key = (core * NT + t_loc) * NQ + q
    order = np.argsort(key, kind="stable")
    skey = key[order]
    ssrc = src[order]
    sdst = dst[order]
    counts = np.bincount(key, minlength=NC * NT * NQ).reshape(NC, NT, NQ)
    nch = (counts.max(axis=0) + P - 1) // P          # [NT, NQ] chunks
    empty = nch.sum(axis=1) == 0
    nch[empty, 0] = 1                                 # every tile gets >=1 chunk

    # stream order: wave emission — st[q0..q2], then (st-1)[q3] deferred
    NST = math.ceil(NT / ST)
    run_order = []
    for st in range(NST):
        for qq in range(min(NQ, 3)):
            run_order.append((st, qq))
        if st >= 1 and NQ == 4:
            run_order.append((st - 1, 3))
    if NQ == 4:
        run_order.append((NST - 1, 3))
    calls = []       # (q, chunk_off, n_chunks) per gather call
    chunk_tile = []  # owning node-tile per chunk, stream order
    chunk_seq = [[[] for _ in range(NQ)] for _ in range(NT)]  # (t,q) -> stream chunk ids
    run_call_end = {}  # (st,q) -> index past the run's last call
    off = 0
    for (st, qq) in run_order:
        tiles = range(st * ST, min((st + 1) * ST, NT))
        call_off = off
        tl = list(tiles)
        for p0 in range(0, len(tl), 2):
            pair = tl[p0:p0 + 2]
            mx = max((int(nch[t, qq]) for t in pair), default=0)
            for k in range(mx):
                for t in pair:
                    if k < nch[t, qq]:
                        chunk_seq[t][qq].append(off)
                        chunk_tile.append(t)
                        off += 1
        co = call_off
        while co < off:
            n = min(MAX_CALL_CH, off - co)
            calls.append((qq, co, n))
            co += n
        run_call_end[(st, qq)] = len(calls)
    TOTCH = off
    # AG2 chunk k triggers after the run that flushes its last z2 tile
    # (q3 of the covering supertile), +2 calls of slack
    ag2_after_call = {}
    if NQ == 4:
        for k in range(3):
            t_last = (min((k + 1) * CPR, NPC) - 1) // P
            st_k = min(t_last // ST, NST - 1)
            ci = min(run_call_end[(st_k, 3)] + 2, len(calls))
            ag2_after_call.setdefault(ci, []).append(k)
    else:
        ag2_after_call = {}
    # AG1 chunk k (k>=1) triggers just before its first consumer run
    ag1_after_call = {}
    if NQ == 4:
        ag1_after_call.setdefault(run_call_end[(0, 0)], []).append(1)
        ag1_after_call.setdefault(run_call_end[(0, 1)], []).append(2)
        ag1_after_call.setdefault(run_call_end[(1, 2)], []).append(3)
    TOT = TOTCH * P
    chunk_tile = np.asarray(chunk_tile)
    # start/stop chunk per tile
    first_chunk = np.full(NT, -1, dtype=np.int64)
    last_chunk = np.full(NT, -1, dtype=np.int64)
    for ci, t in enumerate(chunk_tile):
        if first_chunk[t] < 0:
            first_chunk[t] = ci
        last_chunk[t] = ci

    # ---- per-core padded streams ----
    # chunk-id table [NT, NQ, max_nch] -> stream chunk id
    max_nch = int(nch.max())
    chunk_id_tab = np.zeros((NT, NQ, max_nch), dtype=np.int64)
    for t in range(NT):
        for qq in range(NQ):
            for k, cid in enumerate(chunk_seq[t][qq]):
                chunk_id_tab[t, qq, k] = cid
    group_start = np.searchsorted(skey, np.arange(NC * NT * NQ), side="left")
    rank_in_run = np.arange(len(skey)) - group_start[skey]
    e_t = (sdst - (sdst // NPC) * NPC) // P
    e_rank = ssrc // NPC
    e_loc = ssrc - e_rank * NPC
    e_q = e_loc // CPR
    e_core = sdst // NPC
    pos = chunk_id_tab[e_t, e_q, rank_in_run // P] * P + rank_in_run % P

    idx_stream = np.zeros((NC, TOT), dtype=np.int16)
    dst_stream = np.full((NC, TOT), -1.0, dtype=np.float32)
    idx_stream[e_core, pos] = (e_rank * CPR + e_loc - e_q * CPR).astype(np.int16)
    dst_stream[e_core, pos] = (sdst - e_core * NPC - e_t * P).astype(np.float32)

    # ---- pooling schedule: runs keyed by (core, slot_tile) ----
    seg = seg0
    ncore = npos // NPC
    stile = seg // P
    pkey = ncore * PST + stile
    porder = np.argsort(pkey, kind="stable")
    pskey = pkey[porder]
    pseg = seg[porder]
    pnode_loc = (npos - ncore * NPC)[porder]
    pcounts = np.bincount(pkey, minlength=NC * PST).reshape(NC, PST)
    pnch = (pcounts.max(axis=0) + P - 1) // P
    pnch[pnch == 0] = 1
    prun_off = np.zeros(PST, dtype=np.int64)
    poff = 0
    pchunk_tile = []
    for s in range(PST):
        prun_off[s] = poff
        pchunk_tile.extend([s] * int(pnch[s]))
        poff += int(pnch[s])
    PTOTCH = poff
    PTOT = PTOTCH * P
    pchunk_tile = np.asarray(pchunk_tile)

    pgroup_start = np.searchsorted(pskey, np.arange(NC * PST), side="left")
    prank = np.arange(len(pskey)) - pgroup_start[pskey]
    p_core = pskey // PST
    p_s = pskey % PST
    ppos = prun_off[p_s] * P + prank

    pidx_stream = np.zeros((NC, PTOT), dtype=np.int16)
    pdst_stream = np.full((NC, PTOT), -1.0, dtype=np.float32)
    pidx_stream[p_core, ppos] = pnode_loc.astype(np.int16)
    pdst_stream[p_core, ppos] = (pseg - p_s * P).astype(np.float32)

    cnts = np.bincount(seg, minlength=n_slots).astype(np.float32)
    inv = 1.0 / np.maximum(cnts, 1.0)

    # ---- host-side tensors per core ----
    h_pos = np.zeros((NC * NPC, D), dtype=np.float32)
    h_pos[npos] = h
    hbf = h_pos.astype(bf16_np)
    iota = np.tile(np.arange(P, dtype=np.float32), (P, 1)).astype(bf16_np)
    ones = np.ones((1, P), dtype=np.float32).astype(bf16_np)
    ident = np.eye(P, dtype=np.float32).astype(bf16_np)
    in_maps = []
    for c in range(NC):
        in_maps.append({
            "hT0": np.ascontiguousarray(hbf[c * NPC:(c + 1) * NPC].T),
            "W1": W1.astype(bf16_np),
            "W2": W2.astype(bf16_np),
            "b1": b1.reshape(1, D).astype(bf16_np),
            "b2": b2.reshape(1, D).astype(bf16_np),
            "iota": iota,
            "ones": ones,
            "ident": ident,
            "invc": inv[c * SPC:(c + 1) * SPC].reshape(SPC, 1),
            "idxw": _wrap_idx(idx_stream[c]),
            "dstw": np.ascontiguousarray(
                dst_stream[c].reshape(TOTCH, P).T.astype(bf16_np)),
            "pidxw": _wrap_idx(pidx_stream[c]),
            "pdstw": np.ascontiguousarray(
                pdst_stream[c].reshape(PTOTCH, P).T.astype(bf16_np)),
        })

    s = _Sched()
    s.N, s.D, s.NPC, s.NT, s.NQ, s.NPCP = N, D, NPC, NT, NQ, NPCP
    s.n_slots, s.SPC, s.PST = n_slots, SPC, PST
    s.NST = NST
    s.nch, s.calls, s.chunk_tile = nch, calls, chunk_tile
    s.ag2_after_call = ag2_after_call if NQ == 4 else {}
    s.ag1_after_call = ag1_after_call if NQ == 4 else {}
    s.first_chunk, s.last_chunk = first_chunk, last_chunk
    s.TOTCH, s.TOT = TOTCH, TOT
    s.pnch, s.pchunk_tile, s.PTOTCH, s.PTOT = pnch, pchunk_tile, PTOTCH, PTOT
    s.prun_off = prun_off
    s.max_call_ch = max(n for (_, _, n) in calls)
    s.first_q3_call = next((i for i, (qq, _, _) in enumerate(calls) if qq == 3),
                           len(calls))
    s.CPR = CPR
    return s, in_maps


def _build(s):
    D = s.D
    nc = bacc.Bacc("TRN2", target_bir_lowering=False, debug=False,
                   num_swdge_queues=N_SWQ)

    hT0 = nc.declare_dram_parameter("hT0", [D, s.NPC], BF16, isOutput=False)
    W1 = nc.declare_dram_parameter("W1", [D, D], BF16, isOutput=False)
    W2 = nc.declare_dram_parameter("W2", [D, D], BF16, isOutput=False)
    b1 = nc.declare_dram_parameter("b1", [1, D], BF16, isOutput=False)
    b2 = nc.declare_dram_parameter("b2", [1, D], BF16, isOutput=False)
    iota = nc.declare_dram_parameter("iota", [P, P], BF16, isOutput=False)
    ones = nc.declare_dram_parameter("ones", [1, P], BF16, isOutput=False)
    ident = nc.declare_dram_parameter("ident", [P, P], BF16, isOutput=False)
    invc = nc.declare_dram_parameter("invc", [s.SPC, 1], F32, isOutput=False)
    idxw = nc.declare_dram_parameter("idxw", [P, s.TOT // 16], I16, isOutput=False)
    dstw = nc.declare_dram_parameter("dstw", [P, s.TOTCH], BF16, isOutput=False)
    pidxw = nc.declare_dram_parameter("pidxw", [P, s.PTOT // 16], I16, isOutput=False)
    pdstw = nc.declare_dram_parameter("pdstw", [P, s.PTOTCH], BF16, isOutput=False)
    out_ext = nc.declare_dram_parameter("out", [s.SPC, D], F32, isOutput=True)

    n_tiles = s.NT

    with tile.TileContext(nc) as tc:
        with (
            tc.tile_pool(name="const", bufs=1) as cpool,
            tc.tile_pool(name="hT", bufs=2) as hpool,
            tc.tile_pool(name="idx", bufs=1) as ipool,
            tc.tile_pool(name="gath", bufs=26) as gpool,
            tc.tile_pool(name="oh", bufs=26) as opool,
            tc.tile_pool(name="sml", bufs=6) as spool,
            tc.tile_pool(name="zps", bufs=2, space="PSUM") as zpsum,
            tc.tile_pool(name="scps", bufs=ST, space="PSUM") as scpsum,
            tc.tile_pool(name="dram", bufs=1, space="DRAM") as dram,
        ):
            # ---- constants ----
            iota_t = cpool.tile([P, P], BF16)
            nc.sync.dma_start(out=iota_t[:], in_=iota[:, :])
            ones_t = cpool.tile([1, P], BF16)
            nc.sync.dma_start(out=ones_t[:], in_=ones[:, :])
            ident_t = cpool.tile([P, P], BF16)
            nc.sync.dma_start(out=ident_t[:], in_=ident[:, :])
            w_t = {}
            b_t = {}
            for li, (W, b) in enumerate(((W1, b1), (W2, b2))):
                for k in range(D // P):
                    wt = cpool.tile([P, D], BF16, tag=f"w{li}{k}", name=f"w{li}{k}")
                    w_t[(li, k)] = wt
                    nc.sync.dma_start(out=wt[:], in_=W[k * P:(k + 1) * P, :])
                bt = cpool.tile([1, D], BF16, tag=f"b{li}", name=f"b{li}")
                b_t[li] = bt
                nc.sync.dma_start(out=bt[:], in_=b[:, :])
            invc_t = cpool.tile([s.SPC, 1], F32)
            nc.sync.dma_start(out=invc_t[:], in_=invc[:, :])


            # ---- internal DRAM ----
            z_shard = dram.tile([s.NPC, D], BF16)
            z_full = [[dram.tile([s.CPR * NC, D], BF16, tag=f"zfull{li}_{k}",
                                 name=f"zfull{li}_{k}", addr_space="Shared")
                       for k in range(4)] for li in range(2)]
            h2_dram = dram.tile([s.NPC, D], BF16)
            pool_part = dram.tile([s.n_slots, D], F32)
            pool_red = dram.tile([s.SPC, D], F32)

            def tile_width(t):
                return min(P, s.NPC - t * P)

            swq_state = {"i": 0, "prev": None}

            def issue_gather(out_ap, in_ap, idx_ap, num):
                i = swq_state["i"]
                swq_state["i"] = i + 1
                gi = nc.gpsimd.dma_gather(
                    out_ap, in_ap, idx_ap, num, num, D,
                    queue_num=i % N_SWQ,
                )
                if swq_state["prev"] is not None:
                    add_dep_helper(gi.ins, swq_state["prev"], sync=False,
                                   reason="swdge lane/queue order")
                swq_state["prev"] = gi.ins
                return gi

            def z_tile(li, hT_tiles, t):
                """z = h @ W + b for one node tile -> z_shard rows."""
                tw = tile_width(t)
                zp = zpsum.tile([P, D], F32, tag="zp", space="PSUM",
                                name=f"zp{li}_{t}")
                for k in range(D // P):
                    nc.tensor.matmul(
                        out=zp[:tw],
                        lhsT=hT_tiles[k][:, t * P:t * P + tw],
                        rhs=w_t[(li, k)][:],
                        start=(k == 0),
                        stop=False,
                    )
                nc.tensor.matmul(
                    out=zp[:tw],
                    lhsT=ones_t[:1, :tw],
                    rhs=b_t[li][:],
                    start=False,
                    stop=True,
                )
                zt = spool.tile([P, D], BF16, tag="zt", name=f"zt{li}_{t}")
                nc.vector.tensor_copy(out=zt[:tw], in_=zp[:tw])
                nc.sync.dma_start(out=z_shard[t * P:t * P + tw, :],
                                  in_=zt[:tw])

            def z_phase(li, hT_tiles):
                for t in range(n_tiles):
                    z_tile(li, hT_tiles, t)

            def run_scatter(li, hT_next, call_hook=None):
                zf = z_full[li]
                psum_live = {}

                def flush_tile(t):
                    tw = tile_width(t)
                    ps = psum_live.pop(t)
                    ht = spool.tile([P, D], BF16, tag="ht")
                    nc.scalar.activation(
                        out=ht[:tw], in_=ps[:tw],
                        func=mybir.ActivationFunctionType.Tanh,
                    )
                    if li == 0:
                        # transpose h tile into the next layer's lhsT columns
                        for k in range(D // P):
                            tp = zpsum.tile([P, P], BF16, tag="zp",
                                            space="PSUM", name=f"tp{li}_{t}_{k}")
                            nc.tensor.transpose(
                                out=tp[:, :tw],
                                in_=ht[:tw, k * P:(k + 1) * P],
                                identity=ident_t[:tw, :tw],
                            )
                            nc.vector.tensor_copy(
                                out=hT_next[k][:, t * P:t * P + tw],
                                in_=tp[:, :tw])
                        z_tile(1, hT_next, t)
                    else:
                        nc.sync.dma_start(
                            out=h2_dram[t * P:t * P + tw, :], in_=ht[:tw])

                for call_i, (qq, c_off, n_ch) in enumerate(s.calls):
                    if call_hook is not None:
                        call_hook(call_i)
                    num = n_ch * P
                    g = gpool.tile([P, s.max_call_ch, D], BF16, tag="g")
                    issue_gather(
                        g[:, :n_ch, :],
                        zf[qq][:, :],
                        idx_t[:, c_off * 8:(c_off + n_ch) * 8],
                        num,
                    )
                    oh = opool.tile([P, s.max_call_ch, P], BF16, tag="oh")
                    nc.vector.tensor_tensor(
                        out=oh[:, :n_ch, :],
                        in0=iota_t[:, None, :].to_broadcast([P, n_ch, P]),
                        in1=dst_t[:, c_off:c_off + n_ch, None]
                            .to_broadcast([P, n_ch, P]),
                        op=mybir.AluOpType.is_equal,
                    )
                    for j in range(n_ch):
                        ci = c_off + j
                        t = int(s.chunk_tile[ci])
                        if s.first_chunk[t] == ci:
                            psum_live[t] = scpsum.tile(
                                [P, D], F32, tag="scp", space="PSUM", name=f"scp{t}")
                        nc.tensor.matmul(
                            out=psum_live[t][:],
                            lhsT=oh[:, j, :],
                            rhs=g[:, j, :],
                            start=(s.first_chunk[t] == ci),
                            stop=(s.last_chunk[t] == ci),
                        )
                        if s.last_chunk[t] == ci:
                            flush_tile(t)
                if call_hook is not None:
                    call_hook(len(s.calls))

            # ================= layer 1 =================
            idx_t = ipool.tile([P, s.TOT // 16], I16)
            nc.sync.dma_start(out=idx_t[:], in_=idxw[:, :])
            dst_t = ipool.tile([P, s.TOTCH], BF16)
            nc.sync.dma_start(out=dst_t[:], in_=dstw[:, :])
            pidx_t = ipool.tile([P, s.PTOT // 16], I16)
            nc.sync.dma_start(out=pidx_t[:], in_=pidxw[:, :])
            pdst_t = ipool.tile([P, s.PTOTCH], BF16)
            nc.sync.dma_start(out=pdst_t[:], in_=pdstw[:, :])
            hT_l1 = [hpool.tile([P, s.NPCP], BF16, tag="hT", name=f"hT1_{_k}") for _k in range(D // P)]
            for k in range(D // P):
                nc.sync.dma_start(out=hT_l1[k][:, :s.NPC],
                                  in_=hT0[k * P:(k + 1) * P, :])
            def ag_chunk(li, k):
                nc.gpsimd.collective_compute(
                    "AllGather", mybir.AluOpType.bypass,
                    replica_groups=[list(range(NC))],
                    ins=[z_shard[k * s.CPR:(k + 1) * s.CPR, :].opt()],
                    outs=[z_full[li][k].opt()],
                )

            z_phase(0, hT_l1)
            for k in range(4):
                ag_chunk(0, k)
            ag1_done = {0, 1, 2, 3}
            hT_l2 = [hpool.tile([P, s.NPCP], BF16, tag="hT", name=f"hT2_{_k}") for _k in range(D // P)]

            ag2_done = set()

            def call_hook(ci):
                for k in s.ag1_after_call.get(ci, []):
                    if k not in ag1_done:
                        ag1_done.add(k)
                        ag_chunk(0, k)
                for k in s.ag2_after_call.get(ci, []):
                    if k not in ag2_done:
                        ag2_done.add(k)
                        ag_chunk(1, k)

            run_scatter(0, hT_l2, call_hook=call_hook)
            for k in range(4):
                if k not in ag1_done:
                    ag_chunk(0, k)

            # ================= layer 2 =================
            def call_hook2(ci):
                if ci == s.first_q3_call and 3 not in ag2_done:
                    ag2_done.add(3)
                    ag_chunk(1, 3)

            run_scatter(1, None, call_hook=call_hook2)
            if 3 not in ag2_done:
                ag_chunk(1, 3)

            # ================= pooling =================
            pfirst = np.full(s.PST, -1, dtype=np.int64)
            plast = np.full(s.PST, -1, dtype=np.int64)
            for ci, t in enumerate(s.pchunk_tile):
                if pfirst[t] < 0:
                    pfirst[t] = ci
                plast[t] = ci
            for sl in range(s.PST):
                tot_ch = int(s.pnch[sl])
                base_off = int(s.prun_off[sl])
                pp = scpsum.tile([P, D], F32, tag="scp", space="PSUM")
                done = 0
                while done < tot_ch:
                    n_ch = min(MAX_CALL_CH, tot_ch - done)
                    c_off = base_off + done
                    num = n_ch * P
                    g = gpool.tile([P, s.max_call_ch, D], BF16, tag="g")
                    issue_gather(
                        g[:, :n_ch, :],
                        h2_dram[:s.NPC, :],
                        pidx_t[:, c_off * 8:(c_off + n_ch) * 8],
                        num,
                    )
                    oh = opool.tile([P, s.max_call_ch, P], BF16, tag="oh")
                    nc.vector.tensor_tensor(
                        out=oh[:, :n_ch, :],
                        in0=iota_t[:, None, :].to_broadcast([P, n_ch, P]),
                        in1=pdst_t[:, c_off:c_off + n_ch, None]
                            .to_broadcast([P, n_ch, P]),
                        op=mybir.AluOpType.is_equal,
                    )
                    for j in range(n_ch):
                        nc.tensor.matmul(
                            out=pp[:],
                            lhsT=oh[:, j, :],
                            rhs=g[:, j, :],
                            start=(done + j == 0),
                            stop=(done + j == tot_ch - 1),
                        )
                    done += n_ch
                sw = min(P, s.n_slots - sl * P)
                pc = spool.tile([P, D], F32, tag="pc")
                nc.vector.tensor_copy(out=pc[:sw], in_=pp[:sw])
                nc.sync.dma_start(out=pool_part[sl * P:sl * P + sw, :],
                                  in_=pc[:sw])

            nc.gpsimd.collective_compute(
                "ReduceScatter", mybir.AluOpType.add,
                replica_groups=[list(range(NC))],
                ins=[pool_part.opt()], outs=[pool_red.opt()],
            )
            ot = spool.tile([s.SPC, D], F32, tag="ot")
            nc.sync.dma_start(out=ot[:], in_=pool_red[:, :])
            ot2 = spool.tile([s.SPC, D], F32, tag="ot2")
            nc.vector.tensor_scalar(
                out=ot2[:], in0=ot[:], scalar1=invc_t[:, :1], scalar2=None,
                op0=mybir.AluOpType.mult,
            )
            nc.sync.dma_start(out=out_ext[:, :], in_=ot2[:])

    nc.compile()
    return nc


_CACHE = {}


def _run(inputs, trace=False):
    h = np.asarray(inputs["h"], dtype=np.float32)
    W1 = np.asarray(inputs["W1"], dtype=np.float32)
    b1 = np.asarray(inputs["b1"], dtype=np.float32)
    W2 = np.asarray(inputs["W2"], dtype=np.float32)
    b2 = np.asarray(inputs["b2"], dtype=np.float32)
    edge_src = np.asarray(inputs["edge_src"])
    edge_dst = np.asarray(inputs["edge_dst"])
    seg_ids = np.asarray(inputs["seg_ids"])

    n_slots = 800
    s, in_maps = _preprocess(h, W1, b1, W2, b2, edge_src, edge_dst,
                             seg_ids, n_slots)
    key = (s.N, s.D, s.TOTCH, s.PTOTCH)
    if key not in _CACHE:
        _CACHE[key] = _build(s)
    nc = _CACHE[key]
    if trace:
        _install_ntff_hook()
    try:
        res = run_bass_kernel_spmd(nc, in_maps, core_ids=list(range(NC)),
                                   trace=trace)
    except Exception:
        _axon_reset()
        res = run_bass_kernel_spmd(nc, in_maps, core_ids=list(range(NC)),
                                   trace=trace)
    parts = [res.results[c]["out"] for c in range(NC)]
    full = np.concatenate(parts, axis=0).astype(np.float32)
    return full, res


def kernel(**inputs) -> np.ndarray:
    full, _ = _run(inputs, trace=False)
    return full.reshape(16, 50, full.shape[1])
